# revision 7
# baseline (speedup 1.0000x reference)
"""MinkowskiSwitchNorm Trainium2 kernel v2 (8 NeuronCores, Bass/Tile).

Channels-on-partitions layout: the host sorts points by segment id and packs
each core's 131072 points (2 halves x 65536) into a [128, 65536] bf16 array
whose partition p = half*64 + channel, column j = point index.  Every
8192-column chunk of a half is single-segment.  x is loaded ONCE in bf16 and
stays resident in SBUF.

Pass 1 computes per-chunk column sums with accum_out: the vector engine's
tensor_scalar (4x bf16) produces sum(x); sum(x^2) is split between the scalar
engine (activation Square + accum_out) and DVE (tensor_tensor square +
tensor_scalar accum).  Per-chunk partials are transposed (PE) and combined
per segment with tiny one-hot matmuls, then a 4 KB [8,128] AllReduce merges
cores.  On-chip stats produce per-segment A=inv_std*w, D=b-mean*A; one-hot
matmuls build per-chunk [128,16] A/D tables; pass 2 is a single fused
tensor_scalar per chunk: out = x*A[:,j] + D[:,8+j] in bf16, stored to HBM.
The host upcasts to fp32 and scatters rows back to the original order.
"""

import numpy as np
import ml_dtypes
from contextlib import ExitStack

import concourse.bass as bass
import concourse.tile as tile
from concourse import bacc, mybir
from concourse.bass_utils import run_bass_kernel_spmd

NCORES = 8
B = 8            # segments
C = 64           # channels
NTOT = 1_000_000
P = 128
HALF = 65536             # points per half (= columns per core)
CF = 8192                # columns per chunk
NCH = HALF // CF         # column-chunks per core = 8
CHP = CF                 # points per virtual chunk
TOTCH = NCORES * 2 * NCH  # 128 virtual chunks globally
NLP = 2 * HALF           # padded points per core = 131072
EPS = 1e-5
F32 = mybir.dt.float32
BF16 = mybir.dt.bfloat16

ACT_SX = (0,)              # chunks whose sum(x) runs on the scalar engine
POOL_SX = ()               # gpsimd rejects TensorScalarPtr (engine check)

_CACHE = {}


def _build():
    nc = bacc.Bacc("TRN2", target_bir_lowering=False, debug=False,
                   num_devices=NCORES)

    xt_i = nc.dram_tensor("xt", [P, HALF], BF16, kind="ExternalInput").ap()
    selT_i = nc.dram_tensor("selT", [NCH, 16], BF16,
                            kind="ExternalInput").ap()
    sel2_i = nc.dram_tensor("sel2", [64, 8], F32, kind="ExternalInput").ap()
    sh16_i = nc.dram_tensor("sh16", [8, 40], F32, kind="ExternalInput").ap()
    id128_i = nc.dram_tensor("id128", [P, P], BF16, kind="ExternalInput").ap()
    w_i = nc.dram_tensor("wt", [1, C], F32, kind="ExternalInput").ap()
    b_i = nc.dram_tensor("bs", [1, C], F32, kind="ExternalInput").ap()
    hs_i = nc.dram_tensor("hs", [B, 8], F32, kind="ExternalInput").ap()
    c82_i = nc.dram_tensor("c82", [B, 2], F32, kind="ExternalInput").ap()
    out_o = nc.dram_tensor("out", [P, HALF], BF16, kind="ExternalOutput").ap()

    cc_in = nc.dram_tensor("cc_in", [B, 2 * C], F32)
    cc_out = nc.dram_tensor("cc_out", [B, 2 * C], F32, addr_space="Shared")

    with ExitStack() as ctx:
        tc = ctx.enter_context(tile.TileContext(nc))
        singles = ctx.enter_context(tc.tile_pool(name="singles", bufs=1))
        psumT = ctx.enter_context(tc.tile_pool(name="psT", bufs=1, space="PSUM"))
        psumS = ctx.enter_context(tc.tile_pool(name="psS", bufs=1, space="PSUM"))

        # ---------------- load x (resident, bf16) ----------------
        xc = []
        for j in range(NCH):
            t = singles.tile([P, CF], BF16, name=f"xch{j}")
            nc.sync.dma_start(out=t[:], in_=xt_i[:, j * CF:(j + 1) * CF])
            xc.append(t)

        # ---------------- small constants ----------------
        selT = singles.tile([NCH, 16], BF16)
        nc.scalar.dma_start(out=selT[:], in_=selT_i[:])
        sel2 = singles.tile([64, 8], F32)
        nc.scalar.dma_start(out=sel2[:], in_=sel2_i[:])
        sh16 = singles.tile([8, 40], F32)
        nc.scalar.dma_start(out=sh16[:], in_=sh16_i[:])
        id128 = singles.tile([P, P], BF16)
        nc.scalar.dma_start(out=id128[:], in_=id128_i[:])
        hs = singles.tile([B, 8], F32)
        nc.scalar.dma_start(out=hs[:], in_=hs_i[:])
        c82 = singles.tile([B, 2], F32)
        nc.scalar.dma_start(out=c82[:], in_=c82_i[:])
        w8 = singles.tile([B, C], F32)
        nc.scalar.dma_start(out=w8[:], in_=w_i[:].to_broadcast([B, C]))
        b8 = singles.tile([B, C], F32)
        nc.scalar.dma_start(out=b8[:], in_=b_i[:].to_broadcast([B, C]))

        # zero now; filled after the allreduce (lower-half block at
        # partitions 32:40 -- partition offsets must be 32-aligned)
        UA16 = singles.tile([64, P], F32)
        nc.vector.memset(UA16[:], 0.0)
        UD16 = singles.tile([64, P], F32)
        nc.vector.memset(UD16[:], 0.0)

        # ---------------- pass 1: per-chunk sums ----------------
        # sum(x^2) runs on the scalar engine (Square at 1 elem/cyc with a
        # free accumulator); sum(x) mostly on DVE (TensorScalarPtrReduce is
        # 1x, so full-chunk ops amortize the per-op overhead).  ACT takes
        # the first chunk's sum(x) to balance the engines.
        Pd = singles.tile([P, 16], F32)     # DVE accumulators
        Pa = singles.tile([P, 16], F32)     # ACT accumulators
        Pp = singles.tile([P, 16], F32)     # Pool accumulators
        scrD = singles.tile([P, CF], BF16)  # DVE throwaway out
        scrA = singles.tile([P, CF], BF16)  # ACT throwaway out
        scrP = singles.tile([P, CF], BF16)  # Pool throwaway out

        # preload the sqrt activation table while loads stream
        sqpre = singles.tile([B, 1], F32)
        nc.scalar.activation(out=sqpre[:], in_=hs[:, 7:8],
                             func=mybir.ActivationFunctionType.Sqrt,
                             scale=1.0)

        for j in range(NCH):
            if j in ACT_SX:
                nc.scalar.activation(out=scrA[:], in_=xc[j][:],
                                     func=mybir.ActivationFunctionType.Copy,
                                     accum_out=Pa[:, j:j + 1])
            elif j in POOL_SX:
                nc.gpsimd.tensor_scalar(out=scrP[:], in0=xc[j][:],
                                        scalar1=1.0, scalar2=0.0,
                                        op0=mybir.AluOpType.mult,
                                        op1=mybir.AluOpType.add,
                                        accum_out=Pp[:, j:j + 1])
            else:
                nc.vector.tensor_scalar(out=scrD[:], in0=xc[j][:],
                                        scalar1=1.0, scalar2=0.0,
                                        op0=mybir.AluOpType.mult,
                                        op1=mybir.AluOpType.add,
                                        accum_out=Pd[:, j:j + 1])
            nc.scalar.activation(out=scrA[:], in_=xc[j][:],
                                 func=mybir.ActivationFunctionType.Square,
                                 accum_out=Pa[:, 8 + j:9 + j])

        # gather partials into one bf16 tile (DVE only)
        Pb = singles.tile([P, 16], BF16)
        for j in range(NCH):
            src = Pa if j in ACT_SX else (Pp if j in POOL_SX else Pd)
            nc.vector.tensor_copy(out=Pb[:, j:j + 1], in_=src[:, j:j + 1])
        nc.vector.tensor_copy(out=Pb[:, 8:16], in_=Pa[:, 8:16])

        # transpose partials: two [128, 8] -> [8, 128] (base partition 0)
        psT1 = psumT.tile([NCH, P], BF16)
        nc.tensor.transpose(out=psT1[:], in_=Pb[:, 0:8], identity=id128[:])
        T32a = singles.tile([NCH, P], BF16)   # row j = sum x of chunk j
        nc.vector.tensor_copy(out=T32a[:], in_=psT1[:])
        psT2 = psumT.tile([NCH, P], BF16)
        nc.tensor.transpose(out=psT2[:], in_=Pb[:, 8:16],
                            identity=id128[:])
        T32b = singles.tile([NCH, P], BF16)   # row j = sum x^2
        nc.vector.tensor_copy(out=T32b[:], in_=psT2[:])

        # combine chunks per segment: psS[s, 0:64]=sum x, [64:128]=sum x^2
        psS = psumS.tile([B, 2 * C], F32)
        nc.tensor.matmul(out=psS[:, 0:C], lhsT=selT[:, 0:8],
                         rhs=T32a[:, 0:C], start=True, stop=False)
        nc.tensor.matmul(out=psS[:, 0:C], lhsT=selT[:, 8:16],
                         rhs=T32a[:, C:2 * C], start=False, stop=True)
        nc.tensor.matmul(out=psS[:, C:2 * C], lhsT=selT[:, 0:8],
                         rhs=T32b[:, 0:C], start=True, stop=False)
        nc.tensor.matmul(out=psS[:, C:2 * C], lhsT=selT[:, 8:16],
                         rhs=T32b[:, C:2 * C], start=False, stop=True)
        acc_sb = singles.tile([B, 2 * C], F32)
        nc.vector.tensor_copy(out=acc_sb[:], in_=psS[:])

        # ---------------- all-reduce partials ----------------
        nc.scalar.dma_start(out=cc_in[:], in_=acc_sb[:])
        nc.gpsimd.collective_compute(
            "AllReduce", mybir.AluOpType.add,
            replica_groups=[list(range(NCORES))],
            ins=[cc_in[:]], outs=[cc_out[:]])
        s12 = singles.tile([B, 2 * C], F32)
        nc.scalar.dma_start(out=s12[:], in_=cc_out[:])

        # ---------------- stats -> A/D tables ----------------
        S1g = s12[:, 0:C]
        S2g = s12[:, C:2 * C]
        invc = hs[:, 0:1]

        mean_in = singles.tile([B, C], F32)
        nc.vector.tensor_scalar(out=mean_in[:], in0=S1g, scalar1=invc,
                                scalar2=None, op0=mybir.AluOpType.mult)
        E2 = singles.tile([B, C], F32)
        nc.vector.tensor_scalar(out=E2[:], in0=S2g, scalar1=invc,
                                scalar2=None, op0=mybir.AluOpType.mult)
        var_in = singles.tile([B, C], F32)
        nc.vector.tensor_tensor(out=var_in[:], in0=mean_in[:], in1=mean_in[:],
                                op=mybir.AluOpType.mult)
        nc.vector.tensor_tensor(out=var_in[:], in0=E2[:], in1=var_in[:],
                                op=mybir.AluOpType.subtract)

        mean_ln = singles.tile([B, 1], F32)
        nc.vector.reduce_sum(out=mean_ln[:], in_=mean_in[:],
                             axis=mybir.AxisListType.X)
        nc.vector.tensor_scalar(out=mean_ln[:], in0=mean_ln[:],
                                scalar1=1.0 / C, scalar2=None,
                                op0=mybir.AluOpType.mult)
        E2_ln = singles.tile([B, 1], F32)
        nc.vector.reduce_sum(out=E2_ln[:], in_=E2[:],
                             axis=mybir.AxisListType.X)
        var_ln = singles.tile([B, 1], F32)
        nc.vector.tensor_scalar(out=E2_ln[:], in0=E2_ln[:], scalar1=1.0 / C,
                                scalar2=None, op0=mybir.AluOpType.mult)
        nc.vector.tensor_tensor(out=var_ln[:], in0=mean_ln[:], in1=mean_ln[:],
                                op=mybir.AluOpType.mult)
        nc.vector.tensor_tensor(out=var_ln[:], in0=E2_ln[:], in1=var_ln[:],
                                op=mybir.AluOpType.subtract)

        # column sums over segments (M=1 matmuls, results on partition 0)
        ps_cs = psumS.tile([1, 4 * C], F32)
        nc.tensor.matmul(out=ps_cs[:, 0:2 * C], lhsT=c82[:, 0:1], rhs=s12[:],
                         start=True, stop=True)
        nc.tensor.matmul(out=ps_cs[:, 2 * C:4 * C], lhsT=c82[:, 1:2],
                         rhs=s12[:], start=True, stop=True)
        cs1 = singles.tile([1, 2 * C], F32)
        nc.vector.tensor_copy(out=cs1[:], in_=ps_cs[:, 0:2 * C])
        cs2 = singles.tile([1, 2 * C], F32)
        nc.vector.tensor_copy(out=cs2[:], in_=ps_cs[:, 2 * C:4 * C])
        # mean_bn = cs1[0, 0:C] ;  S2/(N-1) = cs2[0, C:2C]
        mvbn = singles.tile([1, 2 * C], F32)
        nc.vector.tensor_copy(out=mvbn[:, 0:C], in_=cs1[:, 0:C])
        mbn2 = singles.tile([1, C], F32)
        nc.vector.tensor_tensor(out=mbn2[:], in0=cs1[:, 0:C],
                                in1=cs1[:, 0:C], op=mybir.AluOpType.mult)
        nc.vector.tensor_scalar(out=mbn2[:], in0=mbn2[:],
                                scalar1=float(NTOT) / float(NTOT - 1),
                                scalar2=None, op0=mybir.AluOpType.mult)
        nc.vector.tensor_tensor(out=mvbn[:, C:2 * C], in0=cs2[:, C:2 * C],
                                in1=mbn2[:], op=mybir.AluOpType.subtract)

        # broadcast [1,128] -> [8,128] via K=1 matmul with ones
        ones18 = singles.tile([1, B], F32)
        nc.vector.memset(ones18[:], 1.0)
        ps_bc = psumS.tile([B, 2 * C], F32)
        nc.tensor.matmul(out=ps_bc[:], lhsT=ones18[:], rhs=mvbn[:],
                         start=True, stop=True)
        bc = singles.tile([B, 2 * C], F32)
        nc.vector.tensor_copy(out=bc[:], in_=ps_bc[:])

        # mean = mw0*mean_in + mw1*mean_ln + mw2*mean_bn
        mls = singles.tile([B, 1], F32)
        nc.vector.tensor_tensor(out=mls[:], in0=mean_ln[:], in1=hs[:, 2:3],
                                op=mybir.AluOpType.mult)
        mean = singles.tile([B, C], F32)
        nc.vector.tensor_scalar(out=mean[:], in0=mean_in[:],
                                scalar1=hs[:, 1:2], scalar2=mls[:],
                                op0=mybir.AluOpType.mult,
                                op1=mybir.AluOpType.add)
        t2 = singles.tile([B, C], F32)
        nc.vector.tensor_scalar(out=t2[:], in0=bc[:, 0:C], scalar1=hs[:, 3:4],
                                scalar2=None, op0=mybir.AluOpType.mult)
        nc.vector.tensor_tensor(out=mean[:], in0=mean[:], in1=t2[:],
                                op=mybir.AluOpType.add)

        # var = vw0*var_in + vw1*var_ln + vw2*var_bn
        vls = singles.tile([B, 1], F32)
        nc.vector.tensor_tensor(out=vls[:], in0=var_ln[:], in1=hs[:, 5:6],
                                op=mybir.AluOpType.mult)
        var = singles.tile([B, C], F32)
        nc.vector.tensor_scalar(out=var[:], in0=var_in[:],
                                scalar1=hs[:, 4:5], scalar2=vls[:],
                                op0=mybir.AluOpType.mult,
                                op1=mybir.AluOpType.add)
        nc.vector.tensor_scalar(out=t2[:], in0=bc[:, C:2 * C],
                                scalar1=hs[:, 6:7], scalar2=None,
                                op0=mybir.AluOpType.mult)
        nc.vector.tensor_tensor(out=var[:], in0=var[:], in1=t2[:],
                                op=mybir.AluOpType.add)

        # inv_std = 1/sqrt(var+eps);  A = inv_std*w ; D = b - mean*A
        istd = singles.tile([B, C], F32)
        nc.scalar.activation(out=istd[:], in_=var[:],
                             func=mybir.ActivationFunctionType.Sqrt,
                             bias=hs[:, 7:8], scale=1.0)
        nc.vector.reciprocal(out=istd[:], in_=istd[:])
        AD = singles.tile([B, 2 * C], F32)
        nc.vector.tensor_tensor(out=AD[:, 0:C], in0=istd[:], in1=w8[:],
                                op=mybir.AluOpType.mult)
        mA = singles.tile([B, C], F32)
        nc.vector.tensor_tensor(out=mA[:], in0=mean[:], in1=AD[:, 0:C],
                                op=mybir.AluOpType.mult)
        nc.vector.tensor_tensor(out=AD[:, C:2 * C], in0=b8[:], in1=mA[:],
                                op=mybir.AluOpType.subtract)

        # ---------------- per-chunk A/D tables ----------------
        # shift A/D from partitions 0-7 to 8-15 via one-hot matmul
        psSh = psumT.tile([40, 2 * C], F32)
        nc.tensor.matmul(out=psSh[:], lhsT=sh16[:], rhs=AD[:],
                         start=True, stop=True)
        # UA16 rows 0:8 = [A | 0], rows 32:40 = [0 | A]; same for D
        nc.vector.tensor_copy(out=UA16[0:8, 0:C], in_=AD[:, 0:C])
        nc.vector.tensor_copy(out=UA16[32:40, C:2 * C], in_=psSh[32:40, 0:C])
        nc.vector.tensor_copy(out=UD16[0:8, 0:C], in_=AD[:, C:2 * C])
        nc.vector.tensor_copy(out=UD16[32:40, C:2 * C],
                              in_=psSh[32:40, C:2 * C])

        psTab = psumS.tile([P, 16], F32)
        nc.tensor.matmul(out=psTab[:, 0:8], lhsT=UA16[:], rhs=sel2[:],
                         start=True, stop=True)
        nc.tensor.matmul(out=psTab[:, 8:16], lhsT=UD16[:], rhs=sel2[:],
                         start=True, stop=True)
        ADt = singles.tile([P, 16], F32)
        nc.vector.tensor_copy(out=ADt[:], in_=psTab[:])

        # ---------------- pass 2: fused normalize (in place) ----------------
        for j in range(NCH):
            nc.vector.tensor_scalar(
                out=xc[j][:], in0=xc[j][:],
                scalar1=ADt[:, j:j + 1], scalar2=ADt[:, 8 + j:9 + j],
                op0=mybir.AluOpType.mult, op1=mybir.AluOpType.add)
            nc.sync.dma_start(out=out_o[:, j * CF:(j + 1) * CF],
                              in_=xc[j][:])

    nc.compile()
    return nc


def _get_nc():
    if "nc" not in _CACHE:
        _CACHE["nc"] = _build()
    return _CACHE["nc"]


def _softmax32(v):
    v = np.asarray(v, np.float32)
    e = np.exp(v - v.max())
    return (e / e.sum()).astype(np.float32)


def _prep_inputs(x, batch_ids, weight, bias, mean_weight, var_weight):
    x = np.asarray(x, np.float32)
    ids = np.asarray(batch_ids, np.int32)

    counts = np.bincount(ids, minlength=B)
    counts_c = np.maximum(counts, 1)
    mw = _softmax32(mean_weight)
    vw = _softmax32(var_weight)

    hs = np.zeros((B, 8), np.float32)
    hs[:, 0] = (1.0 / counts_c.astype(np.float64)).astype(np.float32)
    hs[:, 1] = mw[0]
    hs[:, 2] = mw[1]
    hs[:, 3] = mw[2]
    hs[:, 4] = vw[0]
    hs[:, 5] = vw[1]
    hs[:, 6] = vw[2]
    hs[:, 7] = EPS
    c82 = np.zeros((B, 2), np.float32)
    c82[:, 0] = 1.0 / NTOT
    c82[:, 1] = 1.0 / (NTOT - 1)
    wt = np.ascontiguousarray(np.asarray(weight, np.float32).reshape(1, C))
    bs = np.ascontiguousarray(np.asarray(bias, np.float32).reshape(1, C))

    # --- sort points by segment; each 8192-point chunk single-segment ---
    order = np.argsort(ids, kind="stable")
    nchunks_b = (counts + CHP - 1) // CHP
    assert nchunks_b.sum() <= TOTCH, "segment sizes exceed chunk capacity"
    chunk_seg = np.full(TOTCH, -1, np.int64)
    seg_chunk_start = np.zeros(B + 1, np.int64)
    pos = 0
    for b in range(B):
        chunk_seg[pos:pos + nchunks_b[b]] = b
        seg_chunk_start[b] = pos
        pos += nchunks_b[b]
    seg_chunk_start[B] = pos

    cum = np.zeros(B + 1, np.int64)
    cum[1:] = np.cumsum(counts)
    ids_sorted = ids[order]
    within = np.arange(NTOT, dtype=np.int64) - cum[ids_sorted]
    dev_slot = seg_chunk_start[ids_sorted] * CHP + within

    xdev = np.zeros((TOTCH * CHP, C), np.float32)
    xdev[dev_slot] = x[order]

    sh16 = np.zeros((8, 40), np.float32)
    sh16[np.arange(8), 32 + np.arange(8)] = 1.0
    id128 = np.eye(P, dtype=ml_dtypes.bfloat16)

    in_maps = []
    for i in range(NCORES):
        blk = xdev[i * 16 * CHP:(i + 1) * 16 * CHP]
        blk = blk.reshape(2, NCH, CHP, C)            # [h, j, t, c]
        xt = np.ascontiguousarray(
            blk.transpose(0, 3, 1, 2).reshape(P, HALF))

        seg_core = chunk_seg[i * 16:(i + 1) * 16].reshape(2, NCH)  # [h, j]
        selU = np.zeros((NCH, B), np.float32)   # [j, s]
        selL = np.zeros((NCH, B), np.float32)
        vu = seg_core[0] >= 0
        selU[np.arange(NCH)[vu], seg_core[0][vu]] = 1.0
        vl = seg_core[1] >= 0
        selL[np.arange(NCH)[vl], seg_core[1][vl]] = 1.0
        selT = np.concatenate([selU, selL], axis=1)      # [8(j), 16]
        sel2 = np.zeros((64, NCH), np.float32)           # rows 0:8 upper,
        sel2[0:8] = selU.T                               # rows 32:40 lower
        sel2[32:40] = selL.T

        in_maps.append(dict(
            xt=np.ascontiguousarray(xt.astype(ml_dtypes.bfloat16)),
            selT=np.ascontiguousarray(selT.astype(ml_dtypes.bfloat16)),
            sel2=sel2, sh16=sh16, id128=id128,
            wt=wt, bs=bs, hs=hs, c82=c82))
    _CACHE["scatter"] = (order, dev_slot)
    return in_maps


def _postprocess(res):
    order, dev_slot = _CACHE["scatter"]
    flat = np.empty((TOTCH * CHP, C), np.float32)
    for i in range(NCORES):
        o = np.asarray(res.results[i]["out"]).astype(np.float32)
        blk = o.reshape(2, C, NCH, CHP).transpose(0, 2, 3, 1)  # [h, j, t, c]
        flat[i * 16 * CHP:(i + 1) * 16 * CHP] = blk.reshape(16 * CHP, C)
    out = np.empty((NTOT, C), np.float32)
    out[order] = flat[dev_slot]
    return out


def kernel(x, batch_ids, weight, bias, mean_weight, var_weight):
    nc = _get_nc()
    in_maps = _prep_inputs(x, batch_ids, weight, bias,
                           mean_weight, var_weight)
    res = run_bass_kernel_spmd(nc, in_maps, list(range(NCORES)))
    _CACHE["last_result"] = res
    return _postprocess(res)
